# revision 7
# baseline (speedup 1.0000x reference)
"""Time-decay LSTM (nn_C_LSTM_15333033247178) Bass kernel for trn2 x8.

Strategy (v1): every core runs the full-batch recurrence (the per-step
matmul streaming cost on the PE is independent of the stationary M dim,
so batch-splitting buys nothing on the matmul side); core m extracts
batch rows [16m, 16m+16) via a per-core selection matmul and writes only
its shard of hidden_seq. Matmuls run as float32r (1 cyc/row at N>=512,
~fp32 precision). Everything is SBUF-resident; gx/td-decay/bias terms are
folded into the PSUM accumulation via one small bf16 matmul per j-tile.

Layout: batch-major. gates[b, j] accumulate in PSUM [128, 512] j-tiles:
  gates = etd5.T @ WQb + sum_k hT[k].T @ U[k]
where etd5 = [e_t, t, t^2, t^3, 1] (per-step, DMA'd pre-transposed from
DRAM) and WQb = [W; Q->f-cols; bias]. h is kept transposed (hT) for the
next step via PE transposes.

SBUF per partition (KB): U_r 128, WQb 8, etd5 2x4, hT 4, c 4, h 4,
i/f 8, tmp 4, stage 2x4, id_r 4, sel_r 4, sout 4  ~= 188 of 192.
"""

import sys

for _p in ("/opt/trn_rl_repo",):
    if _p not in sys.path:
        sys.path.insert(0, _p)

import numpy as np
import ml_dtypes

from concourse import bacc, bass, tile, mybir
from concourse import bass_utils

F32 = mybir.dt.float32
F32R = mybir.dt.float32r
BF16 = mybir.dt.bfloat16

B, S, H = 128, 256, 1024
FH = 4 * H  # 4096
NCORES = 8
BLOC = B // NCORES  # 16
NK = H // 128  # 8 contraction chunks
NJ = FH // 512  # 8 psum j-tiles
ACT = mybir.ActivationFunctionType


def build(repeat: int = 1):
    nc = bacc.Bacc("TRN2", target_bir_lowering=False, debug=False,
                   num_devices=NCORES)

    u_d = nc.dram_tensor("U", [H, FH], F32, kind="ExternalInput")
    wqb_d = nc.dram_tensor("WQb", [16, FH], BF16, kind="ExternalInput")
    etd_d = nc.dram_tensor("etdT", [S, 16, B], BF16, kind="ExternalInput")
    sel_d = nc.dram_tensor("sel", [B, BLOC], F32, kind="ExternalInput")
    ident_d = nc.dram_tensor("ident", [128, 128], F32, kind="ExternalInput")
    hs_d = nc.dram_tensor("hs", [BLOC, S, H], F32, kind="ExternalOutput")
    cs_d = nc.dram_tensor("cs", [BLOC, H], F32, kind="ExternalOutput")

    with tile.TileContext(nc) as tc:
        with (
            tc.tile_pool(name="const", bufs=1) as cpool,
            tc.tile_pool(name="stage", bufs=2) as stpool,
            tc.tile_pool(name="etd", bufs=2) as epool,
            tc.tile_pool(name="sout", bufs=1) as spool,
            tc.tile_pool(name="pg", bufs=3, space="PSUM") as pg,
            tc.tile_pool(name="pt", bufs=1, space="PSUM") as pt,
            tc.tile_pool(name="ps", bufs=1, space="PSUM") as psl,
        ):
            # ---- resident tensors ----
            u_r = cpool.tile([128, NK * FH], F32R)       # 128KB/part
            wqb_sb = cpool.tile([16, FH], BF16)          # 8KB
            sel_r = cpool.tile([B, BLOC], F32R)
            id_r = cpool.tile([128, 128], F32R)
            hT_sb = cpool.tile([128, H], F32R)           # recurrent state h^T
            c_sb = cpool.tile([128, H], F32)             # cell state
            i_sb = cpool.tile([128, H], F32)             # i gate
            f_sb = cpool.tile([128, H], F32)             # f gate
            tmp_sb = cpool.tile([128, H], F32)           # g gate / i*g / tanh(c)
            h_sb = cpool.tile([128, H], F32R)            # o gate, then h

            # ---- load + round constants (f32 -> f32r via DVE) ----
            for k in range(NK):
                for c2 in range(4):
                    st = stpool.tile([128, 1024], F32, tag="stage")
                    nc.sync.dma_start(
                        st[:], u_d[k * 128:(k + 1) * 128,
                                   c2 * 1024:(c2 + 1) * 1024])
                    nc.vector.tensor_copy(
                        u_r[:, k * FH + c2 * 1024:k * FH + (c2 + 1) * 1024],
                        st[:])
            st = stpool.tile([128, 1024], F32, tag="stage")
            nc.sync.dma_start(st[:, 0:BLOC], sel_d[:])
            nc.vector.tensor_copy(sel_r[:], st[:, 0:BLOC])
            st = stpool.tile([128, 1024], F32, tag="stage")
            nc.sync.dma_start(st[:, 0:128], ident_d[:])
            nc.vector.tensor_copy(id_r[:], st[:, 0:128])
            nc.sync.dma_start(wqb_sb[:], wqb_d[:])

            for rep in range(repeat):
                nc.gpsimd.memset(c_sb[:], 0.0)
                for t in range(S):
                    # per-step [16, 128] lhsT: hi/lo split gx operands
                    etd5 = epool.tile([16, B], BF16, tag="etd5")
                    nc.sync.dma_start(etd5[:], etd_d[t, :, :])

                    for jt in range(NJ):
                        g_ps = pg.tile([128, 512], F32, tag="gates")
                        nc.tensor.matmul(
                            g_ps[:], etd5[:],
                            wqb_sb[:, jt * 512:(jt + 1) * 512],
                            start=True, stop=(t == 0),
                        )
                        if t > 0:
                            for k in range(NK):
                                nc.tensor.matmul(
                                    g_ps[:],
                                    hT_sb[:, k * 128:(k + 1) * 128],
                                    u_r[:, k * FH + jt * 512:
                                        k * FH + (jt + 1) * 512],
                                    start=False, stop=(k == NK - 1),
                                )
                        gi, half = jt // 2, jt % 2
                        dst = (i_sb, f_sb, tmp_sb, h_sb)[gi]
                        fn = ACT.Tanh if gi == 2 else ACT.Sigmoid
                        nc.scalar.activation(
                            dst[:, half * 512:(half + 1) * 512], g_ps[:], fn)

                    # c = f*c + i*g ; h = o*tanh(c)   (tmp: g -> i*g -> tanh c)
                    nc.vector.tensor_mul(tmp_sb[:], i_sb[:], tmp_sb[:])
                    nc.vector.tensor_mul(c_sb[:], f_sb[:], c_sb[:])
                    nc.vector.tensor_add(c_sb[:], c_sb[:], tmp_sb[:])
                    nc.scalar.activation(tmp_sb[:], c_sb[:], ACT.Tanh)
                    nc.vector.tensor_mul(h_sb[:], h_sb[:], tmp_sb[:])

                    # hT for the next step
                    hT_ps = pt.tile([128, H], F32R, tag="hT")
                    for k in range(NK):
                        nc.tensor.transpose(
                            hT_ps[:, k * 128:(k + 1) * 128],
                            h_sb[:, k * 128:(k + 1) * 128],
                            id_r[:],
                        )
                    nc.vector.tensor_copy(hT_sb[:], hT_ps[:])

                    # extract this core's batch shard and store
                    s_ps = psl.tile([BLOC, H], F32, tag="sout")
                    nc.tensor.matmul(s_ps[:, 0:512], sel_r[:],
                                     h_sb[:, 0:512], start=True, stop=True)
                    nc.tensor.matmul(s_ps[:, 512:H], sel_r[:],
                                     h_sb[:, 512:H], start=True, stop=True)
                    s_sb = spool.tile([BLOC, H], F32, tag="scp")
                    nc.vector.tensor_copy(s_sb[:], s_ps[:])
                    nc.sync.dma_start(hs_d[:, t, :], s_sb[:])

            # final cell state shard: reuse h_sb (f32r, all writers round)
            nc.vector.tensor_copy(h_sb[:], c_sb[:])
            cs_ps = psl.tile([BLOC, H], F32, tag="sout")
            nc.tensor.matmul(cs_ps[:, 0:512], sel_r[:], h_sb[:, 0:512],
                             start=True, stop=True)
            nc.tensor.matmul(cs_ps[:, 512:H], sel_r[:], h_sb[:, 512:H],
                             start=True, stop=True)
            cs_sb = spool.tile([BLOC, H], F32, tag="scp")
            nc.vector.tensor_copy(cs_sb[:], cs_ps[:])
            nc.sync.dma_start(cs_d[:], cs_sb[:])

    nc.compile()
    return nc


def _host_inputs(events, raw_time_gap, W, U, Q, bias):
    events = np.asarray(events, np.float32)
    rtg = np.asarray(raw_time_gap, np.float32)
    W = np.asarray(W, np.float32)
    U = np.ascontiguousarray(np.asarray(U, np.float32))
    Q = np.asarray(Q, np.float32)
    bias = np.asarray(bias, np.float32)

    # hi/lo bf16 split so each product pair sums to ~fp32 accuracy in PSUM
    def split(x):
        hi = x.astype(ml_dtypes.bfloat16).astype(np.float32)
        lo = (x - hi).astype(ml_dtypes.bfloat16).astype(np.float32)
        return hi, lo

    W_hi, W_lo = split(W[0])
    Q_hi, Q_lo = split(Q)
    b_hi, b_lo = split(bias)
    e = events[:, :, 0].T          # [S, B]
    e_hi, e_lo = split(e)
    # replicate the reference's torch-style concat+reshape scramble exactly
    td_bsk = np.concatenate([rtg, rtg ** 2, rtg ** 3], axis=0).reshape(B, S, 3)
    td = td_bsk.transpose(2, 1, 0)  # [3, S, B]
    td_hi, td_lo = split(td)

    # row pairing: lhsT row r times WQb row r
    wqb = np.zeros((16, FH), np.float32)
    wqb[0] = W_hi
    wqb[1] = W_hi
    wqb[2] = W_lo
    wqb[3:6, H:2 * H] = Q_hi
    wqb[6:9, H:2 * H] = Q_hi
    wqb[9:12, H:2 * H] = Q_lo
    wqb[12] = b_hi
    wqb[13] = b_lo
    wqb = wqb.astype(ml_dtypes.bfloat16)

    etd = np.zeros((S, 16, B), np.float32)
    etd[:, 0, :] = e_hi
    etd[:, 1, :] = e_lo
    etd[:, 2, :] = e_hi
    etd[:, 3:6, :] = td_hi.transpose(1, 0, 2)
    etd[:, 6:9, :] = td_lo.transpose(1, 0, 2)
    etd[:, 9:12, :] = td_hi.transpose(1, 0, 2)
    etd[:, 12, :] = 1.0
    etd[:, 13, :] = 1.0
    etd = etd.astype(ml_dtypes.bfloat16)

    ident = np.eye(128, dtype=np.float32)

    in_maps = []
    for m in range(NCORES):
        sel = np.zeros((B, BLOC), np.float32)
        sel[np.arange(BLOC) + m * BLOC, np.arange(BLOC)] = 1.0
        in_maps.append({
            "U": U, "WQb": wqb, "etdT": etd, "sel": sel, "ident": ident,
        })
    return in_maps


_CACHE = {}


def _run(inputs, repeat=1):
    if repeat not in _CACHE:
        _CACHE[repeat] = build(repeat)
    nc = _CACHE[repeat]
    in_maps = _host_inputs(
        inputs["events"], inputs["raw_time_gap"], inputs["W"],
        inputs["U"], inputs["Q"], inputs["bias"],
    )
    res = bass_utils.run_bass_kernel_spmd(
        nc, in_maps, core_ids=list(range(NCORES)),
    )
    hs = np.concatenate([res.results[m]["hs"] for m in range(NCORES)], axis=0)
    cs = np.concatenate([res.results[m]["cs"] for m in range(NCORES)], axis=0)
    return hs, cs


def kernel(events, raw_time_gap, mask, W, U, Q, bias):
    hs, cs = _run({
        "events": events, "raw_time_gap": raw_time_gap, "W": W, "U": U,
        "Q": Q, "bias": bias,
    })
    h_t = hs[:, -1, :].copy()
    return hs, (h_t, cs)


if __name__ == "__main__":
    rng = np.random.default_rng(0)
    stdv = 1.0 / np.sqrt(H)
    ins = {
        "events": rng.standard_normal((B, S, 1)).astype(np.float32),
        "raw_time_gap": rng.random((B, S)).astype(np.float32),
        "mask": np.ones((B, S), bool),
        "W": rng.uniform(-stdv, stdv, (1, FH)).astype(np.float32),
        "U": rng.uniform(-stdv, stdv, (H, FH)).astype(np.float32),
        "Q": rng.uniform(-stdv, stdv, (3, H)).astype(np.float32),
        "bias": rng.uniform(-stdv, stdv, (FH,)).astype(np.float32),
    }
    out = kernel(**ins)
    print("ok", out[0].shape, out[1][0].shape, out[1][1].shape)


# revision 8
# speedup vs baseline: 1.7621x; 1.7621x over previous
"""Time-decay LSTM (nn_C_LSTM_15333033247178) Bass kernel for trn2 x8.

Strategy (v1): every core runs the full-batch recurrence (the per-step
matmul streaming cost on the PE is independent of the stationary M dim,
so batch-splitting buys nothing on the matmul side); core m extracts
batch rows [16m, 16m+16) via a per-core selection matmul and writes only
its shard of hidden_seq. Matmuls run as float32r (1 cyc/row at N>=512,
~fp32 precision). Everything is SBUF-resident; gx/td-decay/bias terms are
folded into the PSUM accumulation via one small bf16 matmul per j-tile.

Layout: batch-major. gates[b, j] accumulate in PSUM [128, 512] j-tiles:
  gates = etd5.T @ WQb + sum_k hT[k].T @ U[k]
where etd5 = [e_t, t, t^2, t^3, 1] (per-step, DMA'd pre-transposed from
DRAM) and WQb = [W; Q->f-cols; bias]. h is kept transposed (hT) for the
next step via PE transposes.

SBUF per partition (KB): U_r 128, WQb 8, etd5 2x4, hT 4, c 4, h 4,
i/f 8, tmp 4, stage 2x4, id_r 4, sel_r 4, sout 4  ~= 188 of 192.
"""

import sys

for _p in ("/opt/trn_rl_repo",):
    if _p not in sys.path:
        sys.path.insert(0, _p)

import numpy as np
import ml_dtypes

from concourse import bacc, bass, tile, mybir
from concourse import bass_utils

F32 = mybir.dt.float32
F32R = mybir.dt.float32r
BF16 = mybir.dt.bfloat16

B, S, H = 128, 256, 1024
FH = 4 * H  # 4096
NCORES = 8
BLOC = B // NCORES  # 16
NK = H // 128  # 8 contraction chunks
NJ = FH // 512  # 8 psum j-tiles
ACT = mybir.ActivationFunctionType


def build(repeat: int = 1, s_steps: int = S):
    nc = bacc.Bacc("TRN2", target_bir_lowering=False, debug=False,
                   num_devices=NCORES)

    u_d = nc.dram_tensor("U", [H, FH], F32, kind="ExternalInput")
    wqb_d = nc.dram_tensor("WQb", [16, FH], BF16, kind="ExternalInput")
    etd_d = nc.dram_tensor("etdT", [s_steps, 16, B], BF16, kind="ExternalInput")
    sel_d = nc.dram_tensor("sel", [B, BLOC], F32, kind="ExternalInput")
    ident_d = nc.dram_tensor("ident", [128, 128], F32, kind="ExternalInput")
    hs_d = nc.dram_tensor("hs", [BLOC, s_steps, H], F32, kind="ExternalOutput")
    cs_d = nc.dram_tensor("cs", [BLOC, H], F32, kind="ExternalOutput")

    with tile.TileContext(nc) as tc:
        with (
            tc.tile_pool(name="const", bufs=1) as cpool,
            tc.tile_pool(name="stage", bufs=2) as stpool,
            tc.tile_pool(name="etd", bufs=2) as epool,
            tc.tile_pool(name="sout", bufs=1) as spool,
            tc.tile_pool(name="pg", bufs=3, space="PSUM") as pg,
            tc.tile_pool(name="pt", bufs=1, space="PSUM") as pt,
            tc.tile_pool(name="ps", bufs=1, space="PSUM") as psl,
        ):
            # ---- resident tensors ----
            u_r = cpool.tile([128, NK * FH], F32R)       # 128KB/part
            wqb_sb = cpool.tile([16, FH], BF16)          # 8KB
            sel_r = cpool.tile([B, BLOC], F32R)
            id_r = cpool.tile([128, 128], F32R)
            hT_sb = cpool.tile([128, H], F32R)           # recurrent state h^T
            c_sb = cpool.tile([128, H], F32)             # cell state
            i_sb = cpool.tile([128, H], F32)             # i gate
            f_sb = cpool.tile([128, H], F32)             # f gate
            tmp_sb = cpool.tile([128, H], F32)           # g gate / i*g / tanh(c)
            h_sb = cpool.tile([128, H], F32R)            # o gate, then h

            # ---- load + round constants (f32 -> f32r via DVE) ----
            for k in range(NK):
                for c2 in range(4):
                    st = stpool.tile([128, 1024], F32, tag="stage")
                    nc.sync.dma_start(
                        st[:], u_d[k * 128:(k + 1) * 128,
                                   c2 * 1024:(c2 + 1) * 1024])
                    nc.vector.tensor_copy(
                        u_r[:, k * FH + c2 * 1024:k * FH + (c2 + 1) * 1024],
                        st[:])
            st = stpool.tile([128, 1024], F32, tag="stage")
            nc.sync.dma_start(st[:, 0:BLOC], sel_d[:])
            nc.vector.tensor_copy(sel_r[:], st[:, 0:BLOC])
            st = stpool.tile([128, 1024], F32, tag="stage")
            nc.sync.dma_start(st[:, 0:128], ident_d[:])
            nc.vector.tensor_copy(id_r[:], st[:, 0:128])
            nc.sync.dma_start(wqb_sb[:], wqb_d[:])

            for rep in range(repeat):
                nc.gpsimd.memset(c_sb[:], 0.0)
                for t in range(s_steps):
                    # per-step [16, 128] lhsT: hi/lo split gx operands
                    etd5 = epool.tile([16, B], BF16, tag="etd5")
                    nc.sync.dma_start(etd5[:], etd_d[t, :, :])

                    for jt in range(NJ):
                        g_ps = pg.tile([128, 512], F32, tag="gates")
                        nc.tensor.matmul(
                            g_ps[:], etd5[:],
                            wqb_sb[:, jt * 512:(jt + 1) * 512],
                            start=True, stop=(t == 0),
                        )
                        if t > 0:
                            for k in range(NK):
                                nc.tensor.matmul(
                                    g_ps[:],
                                    hT_sb[:, k * 128:(k + 1) * 128],
                                    u_r[:, k * FH + jt * 512:
                                        k * FH + (jt + 1) * 512],
                                    start=False, stop=(k == NK - 1),
                                )
                        gi, half = jt // 2, jt % 2
                        dst = (i_sb, f_sb, tmp_sb, h_sb)[gi]
                        fn = ACT.Tanh if gi == 2 else ACT.Sigmoid
                        nc.scalar.activation(
                            dst[:, half * 512:(half + 1) * 512], g_ps[:], fn)

                    # c = f*c + i*g ; h = o*tanh(c)   (tmp: g -> i*g -> tanh c)
                    nc.vector.tensor_mul(tmp_sb[:], i_sb[:], tmp_sb[:])
                    nc.vector.tensor_mul(c_sb[:], f_sb[:], c_sb[:])
                    nc.vector.tensor_add(c_sb[:], c_sb[:], tmp_sb[:])
                    nc.scalar.activation(tmp_sb[:], c_sb[:], ACT.Tanh)
                    nc.vector.tensor_mul(h_sb[:], h_sb[:], tmp_sb[:])

                    # hT for the next step
                    hT_ps = pt.tile([128, H], F32R, tag="hT")
                    for k in range(NK):
                        nc.tensor.transpose(
                            hT_ps[:, k * 128:(k + 1) * 128],
                            h_sb[:, k * 128:(k + 1) * 128],
                            id_r[:],
                        )
                    nc.vector.tensor_copy(hT_sb[:], hT_ps[:])

                    # extract this core's batch shard and store
                    s_ps = psl.tile([BLOC, H], F32, tag="sout")
                    nc.tensor.matmul(s_ps[:, 0:512], sel_r[:],
                                     h_sb[:, 0:512], start=True, stop=True)
                    nc.tensor.matmul(s_ps[:, 512:H], sel_r[:],
                                     h_sb[:, 512:H], start=True, stop=True)
                    s_sb = spool.tile([BLOC, H], F32, tag="scp")
                    nc.vector.tensor_copy(s_sb[:], s_ps[:])
                    nc.sync.dma_start(hs_d[:, t, :], s_sb[:])

            # final cell state shard: reuse h_sb (f32r, all writers round)
            nc.vector.tensor_copy(h_sb[:], c_sb[:])
            cs_ps = psl.tile([BLOC, H], F32, tag="sout")
            nc.tensor.matmul(cs_ps[:, 0:512], sel_r[:], h_sb[:, 0:512],
                             start=True, stop=True)
            nc.tensor.matmul(cs_ps[:, 512:H], sel_r[:], h_sb[:, 512:H],
                             start=True, stop=True)
            cs_sb = spool.tile([BLOC, H], F32, tag="scp")
            nc.vector.tensor_copy(cs_sb[:], cs_ps[:])
            nc.sync.dma_start(cs_d[:], cs_sb[:])

    nc.compile()
    return nc


def _host_inputs(events, raw_time_gap, W, U, Q, bias):
    events = np.asarray(events, np.float32)
    rtg = np.asarray(raw_time_gap, np.float32)
    W = np.asarray(W, np.float32)
    U = np.ascontiguousarray(np.asarray(U, np.float32))
    Q = np.asarray(Q, np.float32)
    bias = np.asarray(bias, np.float32)

    # hi/lo bf16 split so each product pair sums to ~fp32 accuracy in PSUM
    def split(x):
        hi = x.astype(ml_dtypes.bfloat16).astype(np.float32)
        lo = (x - hi).astype(ml_dtypes.bfloat16).astype(np.float32)
        return hi, lo

    W_hi, W_lo = split(W[0])
    Q_hi, Q_lo = split(Q)
    b_hi, b_lo = split(bias)
    e = events[:, :, 0].T          # [S, B]
    e_hi, e_lo = split(e)
    # replicate the reference's torch-style concat+reshape scramble exactly
    td_bsk = np.concatenate([rtg, rtg ** 2, rtg ** 3], axis=0).reshape(B, S, 3)
    td = td_bsk.transpose(2, 1, 0)  # [3, S, B]
    td_hi, td_lo = split(td)

    # row pairing: lhsT row r times WQb row r
    wqb = np.zeros((16, FH), np.float32)
    wqb[0] = W_hi
    wqb[1] = W_hi
    wqb[2] = W_lo
    wqb[3:6, H:2 * H] = Q_hi
    wqb[6:9, H:2 * H] = Q_hi
    wqb[9:12, H:2 * H] = Q_lo
    wqb[12] = b_hi
    wqb[13] = b_lo
    wqb = wqb.astype(ml_dtypes.bfloat16)

    etd = np.zeros((S, 16, B), np.float32)
    etd[:, 0, :] = e_hi
    etd[:, 1, :] = e_lo
    etd[:, 2, :] = e_hi
    etd[:, 3:6, :] = td_hi.transpose(1, 0, 2)
    etd[:, 6:9, :] = td_lo.transpose(1, 0, 2)
    etd[:, 9:12, :] = td_hi.transpose(1, 0, 2)
    etd[:, 12, :] = 1.0
    etd[:, 13, :] = 1.0
    etd = etd.astype(ml_dtypes.bfloat16)

    ident = np.eye(128, dtype=np.float32)

    in_maps = []
    for m in range(NCORES):
        sel = np.zeros((B, BLOC), np.float32)
        sel[np.arange(BLOC) + m * BLOC, np.arange(BLOC)] = 1.0
        in_maps.append({
            "U": U, "WQb": wqb, "etdT": etd, "sel": sel, "ident": ident,
        })
    return in_maps


_CACHE = {}


def _run(inputs, repeat=1):
    if repeat not in _CACHE:
        _CACHE[repeat] = build(repeat)
    nc = _CACHE[repeat]
    in_maps = _host_inputs(
        inputs["events"], inputs["raw_time_gap"], inputs["W"],
        inputs["U"], inputs["Q"], inputs["bias"],
    )
    res = bass_utils.run_bass_kernel_spmd(
        nc, in_maps, core_ids=list(range(NCORES)),
    )
    hs = np.concatenate([res.results[m]["hs"] for m in range(NCORES)], axis=0)
    cs = np.concatenate([res.results[m]["cs"] for m in range(NCORES)], axis=0)
    return hs, cs


def kernel(events, raw_time_gap, mask, W, U, Q, bias):
    hs, cs = _run({
        "events": events, "raw_time_gap": raw_time_gap, "W": W, "U": U,
        "Q": Q, "bias": bias,
    })
    h_t = hs[:, -1, :].copy()
    return hs, (h_t, cs)


if __name__ == "__main__":
    rng = np.random.default_rng(0)
    stdv = 1.0 / np.sqrt(H)
    ins = {
        "events": rng.standard_normal((B, S, 1)).astype(np.float32),
        "raw_time_gap": rng.random((B, S)).astype(np.float32),
        "mask": np.ones((B, S), bool),
        "W": rng.uniform(-stdv, stdv, (1, FH)).astype(np.float32),
        "U": rng.uniform(-stdv, stdv, (H, FH)).astype(np.float32),
        "Q": rng.uniform(-stdv, stdv, (3, H)).astype(np.float32),
        "bias": rng.uniform(-stdv, stdv, (FH,)).astype(np.float32),
    }
    out = kernel(**ins)
    print("ok", out[0].shape, out[1][0].shape, out[1][1].shape)
